# revision 1
# baseline (speedup 1.0000x reference)
"""Trainium2 Bass kernel for nn_CausalSelfAttention (BitLinear QKV/O + RoPE + causal attn).

Sharding: 2 heads x 2 batches per core (head-parallel). Each core computes its
heads' q/k/v projections (fp32r matmuls at full PE rate), RoPE, causal
flash-style attention in [k, q] score layout with an unnormalized softmax
(denominator via an appended ones column on V), and a column-sharded output
projection producing a partial [4096, 1024] that the host sums across cores.
"""
import sys

sys.path.insert(0, "/opt/trn_rl_repo")

import numpy as np

GROUP = 128
N_HEADS = 16
EPS = 1e-8
B, T, C = 2, 2048, 1024
HD = 64
N_CORES = 8
HPC = N_HEADS // N_CORES  # 2 heads per core


# ---------------------------------------------------------------- host prep
def _ternary_quantize(w):
    O, I = w.shape
    g = w.reshape(O, I // GROUP, GROUP).astype(np.float32)
    scale = np.maximum(np.mean(np.abs(g), axis=-1, keepdims=True), EPS).astype(
        np.float32
    )
    wn = g / scale
    q = np.where(wn > 0.5, 1.0, np.where(wn < -0.5, -1.0, 0.0)).astype(np.float32)
    return (q * scale).reshape(O, I).astype(np.float32)


def _make_core_inputs(x, wq, wk, wv, wo, rope_cos, rope_sin):
    """Returns list of 8 per-core input dicts (float32, device layouts)."""
    x = np.ascontiguousarray(x.astype(np.float32).reshape(B * T, C))
    wq_q = _ternary_quantize(wq)
    wk_q = _ternary_quantize(wk)
    wv_q = _ternary_quantize(wv)
    wo_q = _ternary_quantize(wo)

    xT = x.T  # [1024 c, 4096 t]
    xt_slab = np.ascontiguousarray(
        xT.reshape(8, 128, 8, 512).transpose(2, 1, 0, 3)
    ).astype(np.float32)  # [s, p, cc, u]

    cosT = rope_cos.astype(np.float32).T  # [32, 2048]
    sinT = rope_sin.astype(np.float32).T
    cos_t = np.tile(cosT, (4, 1)).astype(np.float32)
    sin_t = np.concatenate([-sinT, sinT, -sinT, sinT], axis=0).astype(np.float32)
    tri = (np.arange(128)[None, :] >= np.arange(128)[:, None]).astype(np.float32)
    ident = np.eye(128, dtype=np.float32)

    maps = []
    for core in range(N_CORES):
        r0 = core * HPC * HD
        rows = slice(r0, r0 + HPC * HD)

        def w_lhsT(w_qq):
            wsT = w_qq[rows, :].T  # [1024 in, 128 d]
            return np.ascontiguousarray(
                wsT.reshape(8, 128, 128).transpose(1, 0, 2)
            ).astype(np.float32)

        woc = wo_q[:, rows]  # [1024 o, 128 d]
        maps.append(
            {
                "xt": xt_slab,
                "wqT": w_lhsT(wq_q),
                "wkT": w_lhsT(wk_q),
                "wvT": w_lhsT(wv_q),
                "woTA": np.ascontiguousarray(woc[:, :HD].T),
                "woTB": np.ascontiguousarray(woc[:, HD:].T),
                "cos_t": cos_t,
                "sin_t": sin_t,
                "tri": tri,
                "ident": ident,
                "ones65": np.ones((65, 64), np.float32),
                "vinit": np.ones((128, 64 * 65), np.float32),
            }
        )
    return maps


# ---------------------------------------------------------------- BIR post-pass
def _split_excess_waits(nc, max_waits=1):
    """walrus CoreV3 codegen rejects instructions with >1 sem wait; split the
    excess into preceding NoOps on the same engine."""
    import concourse.mybir as mybir

    for f in nc.m.functions:
        for bb in f.blocks:
            insts = bb.instructions
            i = 0
            while i < len(insts):
                ins = insts[i]
                si = ins.sync_info
                if si is not None and si.on_wait and len(si.on_wait) > max_waits:
                    waits = list(si.on_wait)
                    si.on_wait = waits[:max_waits]
                    rest = waits[max_waits:]
                    new_ops = []
                    for j in range(0, len(rest), max_waits):
                        new_ops.append(
                            mybir.InstNoOp(
                                name=nc.get_next_instruction_name(),
                                sync_info=mybir.SyncInfo(
                                    on_wait=rest[j : j + max_waits], on_update=[]
                                ),
                                bass_nofuse=True,
                                engine=ins.engine,
                            )
                        )
                    insts[i:i] = new_ops
                    i += len(new_ops)
                i += 1


# ---------------------------------------------------------------- device kernel
def _emit(nc, tc, d):
    import concourse.mybir as mybir
    from concourse.bass import ds, ts

    f32 = mybir.dt.float32
    f32r = mybir.dt.float32r
    AF = mybir.ActivationFunctionType
    OP = mybir.AluOpType

    with nc.allow_low_precision(reason="fp32r feeds matmuls; fp32 accum in PSUM"), tc.tile_pool(
        name="const", bufs=1
    ) as cp, tc.tile_pool(name="persist", bufs=1) as pp:
        wq_t = cp.tile([128, 8, 128], f32r)
        nc.sync.dma_start(wq_t[:], d["wqT"])
        wk_t = cp.tile([128, 8, 128], f32r)
        nc.sync.dma_start(wk_t[:], d["wkT"])
        wv_t = cp.tile([128, 8, 128], f32r)
        nc.sync.dma_start(wv_t[:], d["wvT"])
        woC = cp.tile([128, 1024], f32r)
        nc.sync.dma_start(woC[0:64, :], d["woTA"])
        nc.sync.dma_start(woC[64:128, :], d["woTB"])
        cos_sb = cp.tile([128, 2048], f32)
        nc.sync.dma_start(cos_sb[:], d["cos_t"])
        sin_sb = cp.tile([128, 2048], f32)
        nc.sync.dma_start(sin_sb[:], d["sin_t"])
        tri_t = cp.tile([128, 128], f32)
        nc.sync.dma_start(tri_t[:], d["tri"])
        id_t = cp.tile([128, 128], f32)
        nc.sync.dma_start(id_t[:], d["ident"])
        ones65 = cp.tile([65, 64], f32r)
        nc.sync.dma_start(ones65[:], d["ones65"])

        qT = pp.tile([128, 4096], f32r)
        kT = pp.tile([128, 4096], f32r)
        v_sb = pp.tile([128, 64 * 65], f32r)
        y2 = pp.tile([128, 4096], f32r)
        y2B = pp.tile([64, 4096], f32r)
        nc.sync.dma_start(v_sb[:], d["vinit"])  # bakes the ones column of v_aug

        # ---- Phase A: projections (fp32r, N=512) + v transpose to [k, d]
        with tc.tile_pool(name="xt", bufs=2) as xtp, tc.tile_pool(
            name="prps", bufs=3, space="PSUM"
        ) as prps, tc.tile_pool(name="vT", bufs=1) as vtp, tc.tile_pool(
            name="tpps", bufs=2, space="PSUM"
        ) as tpps:
            vT = vtp.tile([128, 4096], f32)
            for s in range(8):
                xt_t = xtp.tile([128, 8, 512], f32r)
                nc.sync.dma_start(xt_t[:], d["xt"][s])
                for w_t, dest in ((wq_t, qT), (wk_t, kT), (wv_t, vT)):
                    ps = prps.tile([128, 512], f32)
                    for j in range(8):
                        nc.tensor.matmul(
                            ps[:],
                            w_t[:, j, :],
                            xt_t[:, j, :],
                            start=(j == 0),
                            stop=(j == 7),
                        )
                    nc.vector.tensor_copy(dest[:, ts(s, 512)], ps[:])

            # ---- Phase A2: RoPE on qT, kT (per batch)
            with tc.tile_pool(name="rope", bufs=2) as rp:
                for tns in (qT, kT):
                    for b in range(2):
                        bcols = ds(b * 2048, 2048)
                        sw = rp.tile([128, 2048], f32r, tag="sw")
                        nc.sync.dma_start(sw[0:32, :], tns[32:64, bcols])
                        nc.sync.dma_start(sw[32:64, :], tns[0:32, bcols])
                        nc.sync.dma_start(sw[64:96, :], tns[96:128, bcols])
                        nc.sync.dma_start(sw[96:128, :], tns[64:96, bcols])
                        tmp = rp.tile([128, 2048], f32, tag="tmp")
                        nc.vector.tensor_tensor(
                            tmp[:], tns[:, bcols], cos_sb[:], OP.mult
                        )
                        nc.vector.tensor_tensor(sw[:], sw[:], sin_sb[:], OP.mult)
                        nc.vector.tensor_tensor(tns[:, bcols], tmp[:], sw[:], OP.add)

            # v transposes into v_sb blocks of 65 (col 64 stays 1.0)
            for h in range(2):
                for b in range(2):
                    for j in range(16):
                        tp = tpps.tile([128, 64], f32)
                        idsl = id_t[64 * h : 64 * h + 64, 64 * h : 64 * h + 64]
                        nc.tensor.transpose(
                            tp[:],
                            vT[64 * h : 64 * h + 64, ds(b * 2048 + j * 128, 128)],
                            idsl,
                        )
                        blk = (h * 2 + b) * 16 + j
                        nc.vector.tensor_copy(v_sb[:, ds(blk * 65, 64)], tp[:])

        # ---- Phase B: attention
        with tc.tile_pool(name="eP", bufs=4) as ep, tc.tile_pool(
            name="rcP", bufs=2
        ) as rcp, tc.tile_pool(name="rbP", bufs=2) as rbp, tc.tile_pool(
            name="sps", bufs=3, space="PSUM"
        ) as sps, tc.tile_pool(name="yps", bufs=2, space="PSUM") as yps, tc.tile_pool(
            name="rbps", bufs=1, space="PSUM"
        ) as rbps:
            for b in range(2):
                for qi in range(4):
                    qcols = ds(b * 2048 + qi * 512, 512)
                    nj = 4 * qi + 4
                    yps_h = [yps.tile([65, 512], f32, name="ypA", tag="ypA"),
                             yps.tile([65, 512], f32, name="ypB", tag="ypB")]
                    for j in range(nj):
                        dlt = j * 128 - qi * 512
                        dlt0 = max(dlt, 0)
                        for h in range(2):
                            sp = sps.tile([128, 512], f32)
                            nc.tensor.matmul(
                                sp[:],
                                kT[
                                    64 * h : 64 * h + 64, ds(b * 2048 + j * 128, 128)
                                ],
                                qT[64 * h : 64 * h + 64, qcols],
                                start=True,
                                stop=True,
                            )
                            E = ep.tile([128, 512], f32r)
                            if dlt < 0:
                                nc.scalar.activation(E[:], sp[:], AF.Exp, scale=0.125)
                            else:
                                nc.scalar.activation(
                                    E[:, ds(dlt, 512 - dlt)],
                                    sp[:, ds(dlt, 512 - dlt)],
                                    AF.Exp,
                                    scale=0.125,
                                )
                                nc.vector.tensor_tensor(
                                    E[:, ds(dlt, 128)],
                                    E[:, ds(dlt, 128)],
                                    tri_t[:],
                                    OP.mult,
                                )
                            blk = (h * 2 + b) * 16 + j
                            nc.tensor.matmul(
                                yps_h[h][:, ds(dlt0, 512 - dlt0)],
                                v_sb[:, ds(blk * 65, 65)],
                                E[:, ds(dlt0, 512 - dlt0)],
                                start=(j == 0),
                                stop=(j == nj - 1),
                                skip_group_check=True,
                            )
                    for h in range(2):
                        yp = yps_h[h]
                        rc = rcp.tile([65, 512], f32r)
                        nc.vector.reciprocal(rc[64:65, :], yp[64:65, :])
                        rbq = rbps.tile([64, 512], f32)
                        nc.tensor.matmul(
                            rbq[:],
                            ones65[64:65, :],
                            rc[64:65, :],
                            start=True,
                            stop=True,
                        )
                        rb = rbp.tile([64, 512], f32)
                        nc.vector.tensor_copy(rb[:], rbq[:])
                        dst = y2[0:64, qcols] if h == 0 else y2B[:, qcols]
                        nc.vector.tensor_tensor(dst, yp[0:64, :], rb[:], OP.mult)
                    nc.sync.dma_start(y2[64:128, qcols], y2B[:, qcols])

        # ---- Phase C: output projection (partial over this core's heads)
        with tc.tile_pool(name="obP", bufs=4) as obp, tc.tile_pool(
            name="ops", bufs=2, space="PSUM"
        ) as ops:
            for tcki in range(32):
                for oc in range(2):
                    op = ops.tile([128, 512], f32)
                    nc.tensor.matmul(
                        op[:],
                        y2[:, ts(tcki, 128)],
                        woC[:, ts(oc, 512)],
                        start=True,
                        stop=True,
                    )
                    ob = obp.tile([128, 512], f32)
                    if oc == 0:
                        nc.vector.tensor_copy(ob[:], op[:])
                    else:
                        nc.scalar.copy(ob[:], op[:])
                    nc.sync.dma_start(
                        d["outp"][ds(tcki * 128, 128), ds(oc * 512, 512)], ob[:]
                    )


_NC_CACHE = {}


def _build():
    if "nc" in _NC_CACHE:
        return _NC_CACHE["nc"]
    import concourse.bass as bass
    import concourse.mybir as mybir
    import concourse.tile as tile

    f32 = mybir.dt.float32
    f32r = mybir.dt.float32r
    nc = bass.Bass("TRN2", target_bir_lowering=False, debug=False, num_devices=1)
    d = {
        "xt": nc.dram_tensor("xt", [8, 128, 8, 512], f32r, kind="ExternalInput").ap(),
        "wqT": nc.dram_tensor("wqT", [128, 8, 128], f32r, kind="ExternalInput").ap(),
        "wkT": nc.dram_tensor("wkT", [128, 8, 128], f32r, kind="ExternalInput").ap(),
        "wvT": nc.dram_tensor("wvT", [128, 8, 128], f32r, kind="ExternalInput").ap(),
        "woTA": nc.dram_tensor("woTA", [64, 1024], f32r, kind="ExternalInput").ap(),
        "woTB": nc.dram_tensor("woTB", [64, 1024], f32r, kind="ExternalInput").ap(),
        "cos_t": nc.dram_tensor("cos_t", [128, 2048], f32, kind="ExternalInput").ap(),
        "sin_t": nc.dram_tensor("sin_t", [128, 2048], f32, kind="ExternalInput").ap(),
        "tri": nc.dram_tensor("tri", [128, 128], f32, kind="ExternalInput").ap(),
        "ident": nc.dram_tensor("ident", [128, 128], f32, kind="ExternalInput").ap(),
        "ones65": nc.dram_tensor("ones65", [65, 64], f32r, kind="ExternalInput").ap(),
        "vinit": nc.dram_tensor("vinit", [128, 64 * 65], f32r, kind="ExternalInput").ap(),
        "outp": nc.dram_tensor(
            "outp", [4096, 1024], f32, kind="ExternalOutput"
        ).ap(),
    }
    with tile.TileContext(nc) as tc:
        _emit(nc, tc, d)
    _split_excess_waits(nc)
    _NC_CACHE["nc"] = nc
    return nc


def kernel(x, wq, wk, wv, wo, rope_cos, rope_sin):
    from concourse import bass_utils

    x, wq, wk, wv, wo, rope_cos, rope_sin = (
        np.asarray(a, dtype=np.float32)
        for a in (x, wq, wk, wv, wo, rope_cos, rope_sin)
    )
    in_maps = _make_core_inputs(x, wq, wk, wv, wo, rope_cos, rope_sin)
    nc = _build()
    res = bass_utils.run_bass_kernel_spmd(nc, in_maps, core_ids=list(range(N_CORES)))
    total = np.zeros((B * T, C), np.float32)
    for i in range(N_CORES):
        total += res.results[i]["outp"]
    return total.reshape(B, T, C).astype(np.float32)



# revision 2
# speedup vs baseline: 1.3778x; 1.3778x over previous
"""Trainium2 Bass kernel for nn_CausalSelfAttention (BitLinear QKV/O + RoPE + causal attn).

Sharding: 2 heads x 2 batches per core (head-parallel), bf16 throughout.
Per core: q/k/v projections ([d, t] layout, bf16 matmuls, fp32 PSUM), RoPE via
DMA partition shuffle + DVE/Pool elementwise, scores in [k, q] layout, exp on
ACT (scale=0.125) -> E bf16, pv as many small-N matmuls with E as the
stationary operand giving [q, d_aug] output whose 65th column is the softmax
denominator (ones column baked into v_aug), per-partition normalization via
DVE tensor_tensor with a broadcast reciprocal, PE transposes back to [d, t],
column-sharded output projection producing a bf16 partial [4096, 1024] that
the host sums across cores.

Engine budget per core (TimelineSim model): PE ~102us (at the matmul floor:
cost = out-free-size x 0.417ns/row), ACT ~85us (exp + some copies), DVE ~85us
(copies, rope, epilogue), Pool ~45us (tri masks, rope adds). Emission order
interleaves batch-0 attention with batch-1 projections (and batch-1 attention
with batch-0 out-projection) so exp overlaps projection matmuls.
"""
import sys

sys.path.insert(0, "/opt/trn_rl_repo")

import ml_dtypes
import numpy as np

BF16 = ml_dtypes.bfloat16
GROUP = 128
N_HEADS = 16
EPS = 1e-8
B, T, C = 2, 2048, 1024
HD = 64
N_CORES = 8
HPC = N_HEADS // N_CORES  # 2 heads per core


# ---------------------------------------------------------------- host prep
def _ternary_quantize(w):
    O, I = w.shape
    g = w.reshape(O, I // GROUP, GROUP).astype(np.float32)
    scale = np.maximum(np.mean(np.abs(g), axis=-1, keepdims=True), EPS).astype(
        np.float32
    )
    wn = g / scale
    q = np.where(wn > 0.5, 1.0, np.where(wn < -0.5, -1.0, 0.0)).astype(np.float32)
    return (q * scale).reshape(O, I).astype(np.float32)


def _make_core_inputs(x, wq, wk, wv, wo, rope_cos, rope_sin):
    """Returns list of 8 per-core input dicts (bf16 device layouts)."""
    x = np.ascontiguousarray(x.astype(np.float32).reshape(B * T, C))
    wq_q = _ternary_quantize(wq)
    wk_q = _ternary_quantize(wk)
    wv_q = _ternary_quantize(wv)
    wo_q = _ternary_quantize(wo)

    xT = x.T  # [1024 c, 4096 t]
    xt_slab = np.ascontiguousarray(
        xT.reshape(8, 128, 8, 512).transpose(2, 1, 0, 3)
    ).astype(BF16)  # [s, p, kk, u]

    cosT = rope_cos.astype(np.float32).T  # [32, 2048]
    sinT = rope_sin.astype(np.float32).T
    cos_t = np.tile(cosT, (4, 1)).astype(BF16)
    sin_t = np.concatenate([-sinT, sinT, -sinT, sinT], axis=0).astype(BF16)
    tri = (np.arange(128)[None, :] >= np.arange(128)[:, None]).astype(BF16)
    ident = np.eye(128, dtype=np.float32).astype(BF16)

    maps = []
    for core in range(N_CORES):
        r0 = core * HPC * HD
        rows = slice(r0, r0 + HPC * HD)

        def w_lhsT(w_qq):
            wsT = w_qq[rows, :].T  # [1024 in, 128 d]
            return np.ascontiguousarray(
                wsT.reshape(8, 128, 128).transpose(1, 0, 2)
            ).astype(BF16)  # [p, kk, d]

        woc = wo_q[:, rows]  # [1024 o, 128 d]
        maps.append(
            {
                "xt": xt_slab,
                "wqT": w_lhsT(wq_q),
                "wkT": w_lhsT(wk_q),
                "wvT": w_lhsT(wv_q),
                "woC": np.ascontiguousarray(woc.T).astype(BF16),  # [128 d, 1024 o]
                "cos_t": cos_t,
                "sin_t": sin_t,
                "tri": tri,
                "ident": ident,
            }
        )
    return maps


# ---------------------------------------------------------------- BIR post-pass
def _split_excess_waits(nc, max_waits=1):
    """walrus CoreV3 codegen rejects instructions with >1 sem wait; split the
    excess into preceding NoOps on the same engine."""
    import concourse.mybir as mybir

    for f in nc.m.functions:
        for bb in f.blocks:
            insts = bb.instructions
            i = 0
            while i < len(insts):
                ins = insts[i]
                si = ins.sync_info
                if si is not None and si.on_wait and len(si.on_wait) > max_waits:
                    waits = list(si.on_wait)
                    si.on_wait = waits[:max_waits]
                    rest = waits[max_waits:]
                    new_ops = []
                    for j in range(0, len(rest), max_waits):
                        new_ops.append(
                            mybir.InstNoOp(
                                name=nc.get_next_instruction_name(),
                                sync_info=mybir.SyncInfo(
                                    on_wait=rest[j : j + max_waits], on_update=[]
                                ),
                                bass_nofuse=True,
                                engine=ins.engine,
                            )
                        )
                    insts[i:i] = new_ops
                    i += len(new_ops)
                i += 1


# ---------------------------------------------------------------- device kernel
def _emit(nc, tc, d):
    import concourse.mybir as mybir
    from concourse.bass import ds, ts

    f32 = mybir.dt.float32
    bf16 = mybir.dt.bfloat16
    AF = mybir.ActivationFunctionType
    OP = mybir.AluOpType

    with nc.allow_low_precision(
        reason="bf16 activations; fp32 accum in PSUM; 2e-2 rel tol"
    ), tc.tile_pool(name="const", bufs=1) as cp, tc.tile_pool(
        name="persist", bufs=1
    ) as pp, tc.tile_pool(name="xt", bufs=2) as xtp, tc.tile_pool(
        name="sw", bufs=2
    ) as swp, tc.tile_pool(name="tmp", bufs=2) as tmpp, tc.tile_pool(
        name="E", bufs=3
    ) as epool, tc.tile_pool(name="y2n", bufs=2) as y2np, tc.tile_pool(
        name="rc", bufs=4
    ) as rcp, tc.tile_pool(name="scP", bufs=2, space="PSUM") as scp, tc.tile_pool(
        name="ypP", bufs=2, space="PSUM"
    ) as ypp, tc.tile_pool(name="scrP", bufs=2, space="PSUM") as scr:
        # ---- constants
        wq_t = cp.tile([128, 8, 128], bf16)
        nc.sync.dma_start(wq_t[:], d["wqT"])
        wk_t = cp.tile([128, 8, 128], bf16)
        nc.sync.dma_start(wk_t[:], d["wkT"])
        wv_t = cp.tile([128, 8, 128], bf16)
        nc.sync.dma_start(wv_t[:], d["wvT"])
        woC = cp.tile([128, 1024], bf16)
        nc.sync.dma_start(woC[:], d["woC"])
        cos_sb = cp.tile([128, 2048], bf16)
        nc.sync.dma_start(cos_sb[:], d["cos_t"])
        sin_sb = cp.tile([128, 2048], bf16)
        nc.sync.dma_start(sin_sb[:], d["sin_t"])
        tri_t = cp.tile([128, 128], bf16)
        nc.sync.dma_start(tri_t[:], d["tri"])
        id_t = cp.tile([128, 128], bf16)
        nc.sync.dma_start(id_t[:], d["ident"])

        # ---- persistent tensors
        qT = pp.tile([128, 4096], bf16)
        kT = pp.tile([128, 4096], bf16)
        vT = pp.tile([128, 4096], bf16)
        v_aug = pp.tile([128, 32 * 129], bf16)  # per key-block: [v0(64)|1|v1(64)]
        y2T = pp.tile([128, 4096], bf16)
        bo = [pp.tile([128, 16384], bf16, name=f"bo{b}") for b in range(2)]
        # bake the shared ones columns (col 64 of every 129-block)
        nc.vector.memset(v_aug[:], 1.0)

        # ---- phase A: one projection slab (512 tokens) of q,k,v
        def emit_slab(s):
            xt_t = xtp.tile([128, 8, 512], bf16, tag="xt")
            nc.sync.dma_start(xt_t[:], d["xt"][s])
            for w_t, dest in ((wq_t, qT), (wk_t, kT), (wv_t, vT)):
                ps = scr.tile([128, 512], f32, tag="scr")
                for kk in range(8):
                    nc.tensor.matmul(
                        ps[:],
                        w_t[:, kk, :],
                        xt_t[:, kk, :],
                        start=(kk == 0),
                        stop=(kk == 7),
                    )
                nc.vector.tensor_copy(dest[:, ts(s, 512)], ps[:])

        # ---- RoPE on qT/kT columns of batch b (in place)
        def emit_rope(tns, b, on_pool):
            bcols = ds(b * 2048, 2048)
            sw = swp.tile([128, 2048], bf16, tag="sw")
            nc.sync.dma_start(sw[0:32, :], tns[32:64, bcols])
            nc.sync.dma_start(sw[32:64, :], tns[0:32, bcols])
            nc.sync.dma_start(sw[64:96, :], tns[96:128, bcols])
            nc.sync.dma_start(sw[96:128, :], tns[64:96, bcols])
            tmp = tmpp.tile([128, 2048], bf16, tag="tmp")
            nc.vector.tensor_tensor(tmp[:], tns[:, bcols], cos_sb[:], OP.mult)
            nc.vector.tensor_tensor(sw[:], sw[:], sin_sb[:], OP.mult)
            eng = nc.gpsimd if on_pool else nc.vector
            eng.tensor_tensor(tns[:, bcols], tmp[:], sw[:], OP.add)

        # ---- transpose v into v_aug key-blocks for batch b
        def emit_vtrans(b):
            for blk in range(16):
                g = b * 16 + blk
                tp = scr.tile([128, 128], bf16, tag="scr")
                nc.tensor.transpose(tp[:], vT[:, ds(b * 2048 + blk * 128, 128)], id_t[:])
                nc.vector.tensor_copy(v_aug[:, ds(g * 129, 64)], tp[:, 0:64])
                nc.vector.tensor_copy(v_aug[:, ds(g * 129 + 65, 64)], tp[:, 64:128])

        # ---- phase B: one (batch, 512-query-chunk) of attention
        def emit_chunk(b, qi):
            nj = 4 * qi + 4
            npairs = nj // 2
            yp = [ypp.tile([128, 260], f32, tag="yp", name=f"yp{b}_{qi}_{h}")
                  for h in range(2)]
            q0 = b * 2048 + qi * 512

            units = [(p, h) for p in range(npairs) for h in range(2)]

            def emit_sc(p, h):
                sc = scp.tile([128, 1024], f32, tag="sc", name=f"sc{b}_{qi}_{p}_{h}")
                for jj in range(2):
                    j = 2 * p + jj
                    dlt0 = max(j * 128 - qi * 512, 0)
                    nc.tensor.matmul(
                        sc[:, ds(jj * 512 + dlt0, 512 - dlt0)],
                        kT[64 * h : 64 * h + 64, ds(b * 2048 + j * 128, 128)],
                        qT[64 * h : 64 * h + 64, ds(q0 + dlt0, 512 - dlt0)],
                        start=True,
                        stop=True,
                    )
                return sc

            def emit_rest(p, h, sc):
                # exp (trim the diagonal pairs so unwritten PSUM is never read)
                E = epool.tile([128, 1024], bf16, tag="E")
                j0, j1 = 2 * p, 2 * p + 1
                d0 = max(j0 * 128 - qi * 512, 0)
                d1 = max(j1 * 128 - qi * 512, 0)
                if d0 == 0 and d1 == 0:
                    nc.scalar.activation(E[:], sc[:], AF.Exp, scale=0.125)
                else:
                    nc.scalar.activation(
                        E[:, ds(d0, 512 - d0)], sc[:, ds(d0, 512 - d0)],
                        AF.Exp, scale=0.125,
                    )
                    nc.scalar.activation(
                        E[:, ds(512 + d1, 512 - d1)], sc[:, ds(512 + d1, 512 - d1)],
                        AF.Exp, scale=0.125,
                    )
                # triangular mask on diagonal blocks (Pool engine)
                for jj, j in ((0, j0), (1, j1)):
                    if j >= 4 * qi:
                        qbl = j - 4 * qi
                        esl = E[:, ds(jj * 512 + qbl * 128, 128)]
                        nc.gpsimd.tensor_tensor(esl, esl, tri_t[:], OP.mult)
                # pv: E block is the stationary operand -> out [q, d_aug]
                for jj, j in ((0, j0), (1, j1)):
                    g = b * 16 + j
                    for qbl in range(4):
                        qb_g = qi * 4 + qbl
                        if qb_g < j:
                            continue
                        nc.tensor.matmul(
                            yp[h][:, ds(qbl * 65, 65)],
                            E[:, ds(jj * 512 + qbl * 128, 128)],
                            v_aug[:, ds(g * 129 + 64 * h, 65)],
                            start=(j == 0),
                            stop=(j == qb_g),
                            skip_group_check=True,
                        )

            prev = None
            for u in units:
                sc = emit_sc(*u)
                if prev is not None:
                    emit_rest(prev[0], prev[1], prev[2])
                prev = (u[0], u[1], sc)
            emit_rest(prev[0], prev[1], prev[2])

            # epilogue: normalize by the denominator column, then transpose
            y2n = y2np.tile([128, 512], bf16, tag="y2n")
            for h in range(2):
                ypr = yp[h][:].rearrange("p (a c) -> p a c", a=4)
                rc = rcp.tile([128, 4], f32, tag="rc")
                den = ypr[:, :, 64:65] if h == 0 else ypr[:, :, 0:1]
                nc.vector.reciprocal(rc[:], den)
                data = ypr[:, :, 0:64] if h == 0 else ypr[:, :, 1:65]
                rcb = rc[:].unsqueeze(2).broadcast_to([128, 4, 64])
                outap = y2n[:].rearrange("p (a c) -> p a c", a=4)[
                    :, :, 64 * h : 64 * h + 64
                ]
                nc.vector.tensor_tensor(outap, data, rcb, OP.mult)
            for qbl in range(4):
                tp = scr.tile([128, 128], bf16, tag="scr")
                nc.tensor.transpose(tp[:], y2n[:, ds(qbl * 128, 128)], id_t[:])
                nc.vector.tensor_copy(
                    y2T[:, ds(b * 2048 + (qi * 4 + qbl) * 128, 128)], tp[:]
                )

        # ---- phase C: output projection for 4 token-blocks + one out DMA
        _copy_ctr = [0]

        def emit_outgroup(b, grp):
            for tbl in range(4):
                tb = grp * 4 + tbl
                for oc in range(2):
                    op = scr.tile([128, 512], f32, tag="scr")
                    nc.tensor.matmul(
                        op[:],
                        y2T[:, ds(b * 2048 + tb * 128, 128)],
                        woC[:, ds(oc * 512, 512)],
                        start=True,
                        stop=True,
                    )
                    dst = bo[b][:, ds(tb * 1024 + oc * 512, 512)]
                    _copy_ctr[0] += 1
                    if _copy_ctr[0] % 4 == 0:
                        nc.scalar.copy(dst, op[:])
                    else:
                        nc.vector.tensor_copy(dst, op[:])
            dram = d["outp"][ds(b * 2048 + grp * 512, 512), :].rearrange(
                "(a p) c -> p a c", p=128
            )
            src = bo[b][:, ds(grp * 4096, 4096)].rearrange("p (a c) -> p a c", a=4)
            nc.sync.dma_start(dram, src)

        # ---------------- emission schedule ----------------
        for s in range(4):
            emit_slab(s)
        emit_rope(qT, 0, on_pool=False)
        emit_rope(kT, 0, on_pool=False)
        emit_vtrans(0)
        emit_slab(4)
        emit_slab(5)
        emit_chunk(0, 0)
        emit_slab(6)
        emit_chunk(0, 1)
        emit_slab(7)
        emit_rope(qT, 1, on_pool=True)
        emit_rope(kT, 1, on_pool=True)
        emit_chunk(0, 2)
        emit_vtrans(1)
        emit_chunk(0, 3)
        emit_chunk(1, 0)
        emit_outgroup(0, 0)
        emit_chunk(1, 1)
        emit_outgroup(0, 1)
        emit_chunk(1, 2)
        emit_outgroup(0, 2)
        emit_chunk(1, 3)
        emit_outgroup(0, 3)
        for grp in range(4):
            emit_outgroup(1, grp)


_NC_CACHE = {}


def _build():
    if "nc" in _NC_CACHE:
        return _NC_CACHE["nc"]
    import concourse.bass as bass
    import concourse.mybir as mybir
    import concourse.tile as tile

    bf16 = mybir.dt.bfloat16
    nc = bass.Bass("TRN2", target_bir_lowering=False, debug=False, num_devices=1)
    d = {
        "xt": nc.dram_tensor("xt", [8, 128, 8, 512], bf16, kind="ExternalInput").ap(),
        "wqT": nc.dram_tensor("wqT", [128, 8, 128], bf16, kind="ExternalInput").ap(),
        "wkT": nc.dram_tensor("wkT", [128, 8, 128], bf16, kind="ExternalInput").ap(),
        "wvT": nc.dram_tensor("wvT", [128, 8, 128], bf16, kind="ExternalInput").ap(),
        "woC": nc.dram_tensor("woC", [128, 1024], bf16, kind="ExternalInput").ap(),
        "cos_t": nc.dram_tensor("cos_t", [128, 2048], bf16, kind="ExternalInput").ap(),
        "sin_t": nc.dram_tensor("sin_t", [128, 2048], bf16, kind="ExternalInput").ap(),
        "tri": nc.dram_tensor("tri", [128, 128], bf16, kind="ExternalInput").ap(),
        "ident": nc.dram_tensor("ident", [128, 128], bf16, kind="ExternalInput").ap(),
        "outp": nc.dram_tensor("outp", [4096, 1024], bf16, kind="ExternalOutput").ap(),
    }
    with tile.TileContext(nc) as tc:
        _emit(nc, tc, d)
    _split_excess_waits(nc)
    _NC_CACHE["nc"] = nc
    return nc


def kernel(x, wq, wk, wv, wo, rope_cos, rope_sin):
    from concourse import bass_utils

    x, wq, wk, wv, wo, rope_cos, rope_sin = (
        np.asarray(a, dtype=np.float32)
        for a in (x, wq, wk, wv, wo, rope_cos, rope_sin)
    )
    in_maps = _make_core_inputs(x, wq, wk, wv, wo, rope_cos, rope_sin)
    nc = _build()
    res = bass_utils.run_bass_kernel_spmd(nc, in_maps, core_ids=list(range(N_CORES)))
    total = np.zeros((B * T, C), np.float32)
    for i in range(N_CORES):
        total += res.results[i]["outp"].astype(np.float32)
    return total.reshape(B, T, C).astype(np.float32)


# revision 4
# speedup vs baseline: 1.4390x; 1.0444x over previous
"""Trainium2 Bass kernel for nn_CausalSelfAttention (BitLinear QKV/O + RoPE + causal attn).

Sharding: 2 heads x 2 batches per core (head-parallel), bf16 throughout.
Per core: q/k/v projections ([d, t] layout, bf16 matmuls, fp32 PSUM), RoPE via
DMA partition shuffle + DVE/Pool elementwise, scores in [k, q] layout, exp on
ACT (scale=0.125) -> E bf16, pv as many small-N matmuls with E as the
stationary operand giving [q, d_aug] output whose 65th column is the softmax
denominator (ones column baked into v_aug), per-partition normalization via
DVE tensor_tensor with a broadcast reciprocal, PE transposes back to [d, t],
column-sharded output projection producing a bf16 partial [4096, 1024] that
the host sums across cores.

Engine budget per core (TimelineSim model): PE ~102us (at the matmul floor:
cost = out-free-size x 0.417ns/row), ACT ~85us (exp + some copies), DVE ~85us
(copies, rope, epilogue), Pool ~45us (tri masks, rope adds). Emission order
interleaves batch-0 attention with batch-1 projections (and batch-1 attention
with batch-0 out-projection) so exp overlaps projection matmuls.
"""
import sys

sys.path.insert(0, "/opt/trn_rl_repo")

import ml_dtypes
import numpy as np

BF16 = ml_dtypes.bfloat16
GROUP = 128
N_HEADS = 16
EPS = 1e-8
B, T, C = 2, 2048, 1024
HD = 64
N_CORES = 8
HPC = N_HEADS // N_CORES  # 2 heads per core


# ---------------------------------------------------------------- host prep
def _ternary_quantize(w):
    O, I = w.shape
    g = w.reshape(O, I // GROUP, GROUP).astype(np.float32)
    scale = np.maximum(np.mean(np.abs(g), axis=-1, keepdims=True), EPS).astype(
        np.float32
    )
    wn = g / scale
    q = np.where(wn > 0.5, 1.0, np.where(wn < -0.5, -1.0, 0.0)).astype(np.float32)
    return (q * scale).reshape(O, I).astype(np.float32)


def _make_core_inputs(x, wq, wk, wv, wo, rope_cos, rope_sin):
    """Returns list of 8 per-core input dicts (bf16 device layouts)."""
    x = np.ascontiguousarray(x.astype(np.float32).reshape(B * T, C))
    wq_q = _ternary_quantize(wq)
    wk_q = _ternary_quantize(wk)
    wv_q = _ternary_quantize(wv)
    wo_q = _ternary_quantize(wo)

    xT = x.T  # [1024 c, 4096 t]
    xt_slab = np.ascontiguousarray(
        xT.reshape(8, 128, 8, 512).transpose(2, 1, 0, 3)
    ).astype(BF16)  # [s, p, kk, u]

    cosT = rope_cos.astype(np.float32).T  # [32, 2048]
    sinT = rope_sin.astype(np.float32).T
    cos_t = np.tile(cosT, (4, 1)).astype(BF16)
    sin_t = np.concatenate([-sinT, sinT, -sinT, sinT], axis=0).astype(BF16)
    tri = (np.arange(128)[None, :] >= np.arange(128)[:, None]).astype(BF16)
    ident = np.eye(128, dtype=np.float32).astype(BF16)

    maps = []
    for core in range(N_CORES):
        r0 = core * HPC * HD
        rows = slice(r0, r0 + HPC * HD)

        def w_lhsT(w_qq):
            wsT = w_qq[rows, :].T  # [1024 in, 128 d]
            return np.ascontiguousarray(
                wsT.reshape(8, 128, 128).transpose(1, 0, 2)
            ).astype(BF16)  # [p, kk, d]

        woc = wo_q[:, rows]  # [1024 o, 128 d]
        maps.append(
            {
                "xt": xt_slab,
                "wqT": w_lhsT(wq_q),
                "wkT": w_lhsT(wk_q),
                "wvT": w_lhsT(wv_q),
                "woC": np.ascontiguousarray(woc.T).astype(BF16),  # [128 d, 1024 o]
                "cos_t": cos_t,
                "sin_t": sin_t,
                "tri": tri,
                "ident": ident,
            }
        )
    return maps


# ---------------------------------------------------------------- BIR post-pass
def _split_excess_waits(nc, max_waits=1):
    """walrus CoreV3 codegen rejects instructions with >1 sem wait; split the
    excess into preceding NoOps on the same engine."""
    import concourse.mybir as mybir

    for f in nc.m.functions:
        for bb in f.blocks:
            insts = bb.instructions
            i = 0
            while i < len(insts):
                ins = insts[i]
                si = ins.sync_info
                if si is not None and si.on_wait and len(si.on_wait) > max_waits:
                    waits = list(si.on_wait)
                    si.on_wait = waits[:max_waits]
                    rest = waits[max_waits:]
                    new_ops = []
                    for j in range(0, len(rest), max_waits):
                        new_ops.append(
                            mybir.InstNoOp(
                                name=nc.get_next_instruction_name(),
                                sync_info=mybir.SyncInfo(
                                    on_wait=rest[j : j + max_waits], on_update=[]
                                ),
                                bass_nofuse=True,
                                engine=ins.engine,
                            )
                        )
                    insts[i:i] = new_ops
                    i += len(new_ops)
                i += 1


# ---------------------------------------------------------------- device kernel
def _emit(nc, tc, d):
    import concourse.mybir as mybir
    from concourse.bass import ds, ts

    f32 = mybir.dt.float32
    bf16 = mybir.dt.bfloat16
    AF = mybir.ActivationFunctionType
    OP = mybir.AluOpType

    with nc.allow_low_precision(
        reason="bf16 activations; fp32 accum in PSUM; 2e-2 rel tol"
    ), tc.tile_pool(name="const", bufs=1) as cp, tc.tile_pool(
        name="persist", bufs=1
    ) as pp, tc.tile_pool(name="xt", bufs=2) as xtp, tc.tile_pool(
        name="sw", bufs=2
    ) as swp, tc.tile_pool(name="tmp", bufs=2) as tmpp, tc.tile_pool(
        name="E", bufs=3
    ) as epool, tc.tile_pool(name="y2n", bufs=2) as y2np, tc.tile_pool(
        name="rc", bufs=4
    ) as rcp, tc.tile_pool(name="scP", bufs=2, space="PSUM") as scp, tc.tile_pool(
        name="ypP", bufs=2, space="PSUM"
    ) as ypp, tc.tile_pool(name="scrP", bufs=2, space="PSUM") as scr:
        # ---- constants
        wq_t = cp.tile([128, 8, 128], bf16)
        nc.sync.dma_start(wq_t[:], d["wqT"])
        wk_t = cp.tile([128, 8, 128], bf16)
        nc.sync.dma_start(wk_t[:], d["wkT"])
        wv_t = cp.tile([128, 8, 128], bf16)
        nc.sync.dma_start(wv_t[:], d["wvT"])
        woC = cp.tile([128, 1024], bf16)
        nc.sync.dma_start(woC[:], d["woC"])
        cos_sb = cp.tile([128, 2048], bf16)
        nc.sync.dma_start(cos_sb[:], d["cos_t"])
        sin_sb = cp.tile([128, 2048], bf16)
        nc.sync.dma_start(sin_sb[:], d["sin_t"])
        tri_t = cp.tile([128, 128], bf16)
        nc.sync.dma_start(tri_t[:], d["tri"])
        id_t = cp.tile([128, 128], bf16)
        nc.sync.dma_start(id_t[:], d["ident"])

        # ---- persistent tensors
        qT = pp.tile([128, 4096], bf16)
        kT = pp.tile([128, 4096], bf16)
        vT = pp.tile([128, 4096], bf16)
        v_aug = pp.tile([128, 32 * 129], bf16)  # per key-block: [v0(64)|1|v1(64)]
        y2T = pp.tile([128, 4096], bf16)
        bo = [pp.tile([128, 16384], bf16, name=f"bo{b}") for b in range(2)]
        # bake the shared ones columns (col 64 of every 129-block)
        nc.gpsimd.memset(v_aug[:], 1.0)

        # ---- phase A: one projection slab (512 tokens) of q,k,v; then RoPE
        # on the slab's q/k columns and transpose its v into v_aug — so
        # attention on this slab's queries can start immediately after.
        def emit_slab(s, qk_on_act):
            xt_t = xtp.tile([128, 8, 512], bf16, tag="xt")
            nc.sync.dma_start(xt_t[:], d["xt"][s])
            for w_t, dest in ((wq_t, qT), (wk_t, kT), (wv_t, vT)):
                ps = scr.tile([128, 512], f32, tag="scr")
                for kk in range(8):
                    nc.tensor.matmul(
                        ps[:],
                        w_t[:, kk, :],
                        xt_t[:, kk, :],
                        start=(kk == 0),
                        stop=(kk == 7),
                    )
                if qk_on_act and dest is not vT:
                    nc.scalar.copy(dest[:, ts(s, 512)], ps[:])
                else:
                    nc.vector.tensor_copy(dest[:, ts(s, 512)], ps[:])
            # RoPE (in place) on this slab's q/k columns
            u = (s % 4) * 512  # within-batch token offset
            ccols = ds(u, 512)
            scols = ds(s * 512, 512)
            for tns in (qT, kT):
                sw = swp.tile([128, 512], bf16, tag="sw")
                nc.sync.dma_start(sw[0:32, :], tns[32:64, scols])
                nc.sync.dma_start(sw[32:64, :], tns[0:32, scols])
                nc.sync.dma_start(sw[64:96, :], tns[96:128, scols])
                nc.sync.dma_start(sw[96:128, :], tns[64:96, scols])
                tmp = tmpp.tile([128, 512], bf16, tag="tmp")
                nc.vector.tensor_tensor(tmp[:], tns[:, scols], cos_sb[:, ccols], OP.mult)
                nc.vector.tensor_tensor(sw[:], sw[:], sin_sb[:, ccols], OP.mult)
                nc.gpsimd.tensor_tensor(tns[:, scols], tmp[:], sw[:], OP.add)
            # v transposes for this slab's 4 key-blocks
            for blk in range(4):
                g = s * 4 + blk
                tp = scr.tile([128, 128], bf16, tag="scr")
                nc.tensor.transpose(tp[:], vT[:, ds(g * 128, 128)], id_t[:])
                nc.vector.tensor_copy(v_aug[:, ds(g * 129, 64)], tp[:, 0:64])
                nc.vector.tensor_copy(v_aug[:, ds(g * 129 + 65, 64)], tp[:, 64:128])

        # ---- phase B: one (batch, 512-query-chunk) of attention
        def emit_chunk(b, qi):
            nj = 4 * qi + 4
            npairs = nj // 2
            yp = [ypp.tile([128, 260], f32, tag="yp", name=f"yp{b}_{qi}_{h}")
                  for h in range(2)]
            q0 = b * 2048 + qi * 512

            units = [(p, h) for p in range(npairs) for h in range(2)]

            def emit_sc(p, h):
                sc = scp.tile([128, 1024], f32, tag="sc", name=f"sc{b}_{qi}_{p}_{h}")
                for jj in range(2):
                    j = 2 * p + jj
                    dlt0 = max(j * 128 - qi * 512, 0)
                    nc.tensor.matmul(
                        sc[:, ds(jj * 512 + dlt0, 512 - dlt0)],
                        kT[64 * h : 64 * h + 64, ds(b * 2048 + j * 128, 128)],
                        qT[64 * h : 64 * h + 64, ds(q0 + dlt0, 512 - dlt0)],
                        start=True,
                        stop=True,
                    )
                return sc

            def emit_rest(p, h, sc):
                # exp (trim the diagonal pairs so unwritten PSUM is never read)
                E = epool.tile([128, 1024], bf16, tag="E")
                j0, j1 = 2 * p, 2 * p + 1
                d0 = max(j0 * 128 - qi * 512, 0)
                d1 = max(j1 * 128 - qi * 512, 0)
                if d0 == 0 and d1 == 0:
                    nc.scalar.activation(E[:], sc[:], AF.Exp, scale=0.125)
                else:
                    nc.scalar.activation(
                        E[:, ds(d0, 512 - d0)], sc[:, ds(d0, 512 - d0)],
                        AF.Exp, scale=0.125,
                    )
                    nc.scalar.activation(
                        E[:, ds(512 + d1, 512 - d1)], sc[:, ds(512 + d1, 512 - d1)],
                        AF.Exp, scale=0.125,
                    )
                # triangular mask on diagonal blocks (Pool engine)
                for jj, j in ((0, j0), (1, j1)):
                    if j >= 4 * qi:
                        qbl = j - 4 * qi
                        esl = E[:, ds(jj * 512 + qbl * 128, 128)]
                        nc.gpsimd.tensor_tensor(esl, esl, tri_t[:], OP.mult)
                # pv: E block is the stationary operand -> out [q, d_aug]
                for jj, j in ((0, j0), (1, j1)):
                    g = b * 16 + j
                    for qbl in range(4):
                        qb_g = qi * 4 + qbl
                        if qb_g < j:
                            continue
                        nc.tensor.matmul(
                            yp[h][:, ds(qbl * 65, 65)],
                            E[:, ds(jj * 512 + qbl * 128, 128)],
                            v_aug[:, ds(g * 129 + 64 * h, 65)],
                            start=(j == 0),
                            stop=(j == qb_g),
                            skip_group_check=True,
                        )

            prev = None
            for u in units:
                sc = emit_sc(*u)
                if prev is not None:
                    emit_rest(prev[0], prev[1], prev[2])
                prev = (u[0], u[1], sc)
            emit_rest(prev[0], prev[1], prev[2])

            # epilogue: normalize by the denominator column, then transpose
            y2n = y2np.tile([128, 512], bf16, tag="y2n")
            for h in range(2):
                ypr = yp[h][:].rearrange("p (a c) -> p a c", a=4)
                rc = rcp.tile([128, 4], f32, tag="rc")
                den = ypr[:, :, 64:65] if h == 0 else ypr[:, :, 0:1]
                nc.vector.reciprocal(rc[:], den)
                data = ypr[:, :, 0:64] if h == 0 else ypr[:, :, 1:65]
                rcb = rc[:].unsqueeze(2).broadcast_to([128, 4, 64])
                outap = y2n[:].rearrange("p (a c) -> p a c", a=4)[
                    :, :, 64 * h : 64 * h + 64
                ]
                nc.vector.tensor_tensor(outap, data, rcb, OP.mult)
            for qbl in range(4):
                tp = scr.tile([128, 128], bf16, tag="scr")
                nc.tensor.transpose(tp[:], y2n[:, ds(qbl * 128, 128)], id_t[:])
                nc.vector.tensor_copy(
                    y2T[:, ds(b * 2048 + (qi * 4 + qbl) * 128, 128)], tp[:]
                )

        # ---- phase C: output projection for 4 token-blocks + one out DMA
        _copy_ctr = [0]

        def emit_outgroup(b, grp, act_mod=4):
            for tbl in range(4):
                tb = grp * 4 + tbl
                for oc in range(2):
                    op = scr.tile([128, 512], f32, tag="scr")
                    nc.tensor.matmul(
                        op[:],
                        y2T[:, ds(b * 2048 + tb * 128, 128)],
                        woC[:, ds(oc * 512, 512)],
                        start=True,
                        stop=True,
                    )
                    dst = bo[b][:, ds(tb * 1024 + oc * 512, 512)]
                    _copy_ctr[0] += 1
                    if _copy_ctr[0] % act_mod == 0:
                        nc.scalar.copy(dst, op[:])
                    else:
                        nc.vector.tensor_copy(dst, op[:])
            dram = d["outp"][ds(b * 2048 + grp * 512, 512), :].rearrange(
                "(a p) c -> p a c", p=128
            )
            src = bo[b][:, ds(grp * 4096, 4096)].rearrange("p (a c) -> p a c", a=4)
            nc.sync.dma_start(dram, src)

        # ---------------- emission schedule ----------------
        # slab s feeds chunk (s//4, s%4); attention starts right after slab 0.
        emit_slab(0, qk_on_act=True)
        emit_slab(1, qk_on_act=True)
        emit_chunk(0, 0)
        emit_slab(2, qk_on_act=False)
        emit_chunk(0, 1)
        emit_slab(3, qk_on_act=False)
        emit_chunk(0, 2)
        emit_slab(4, qk_on_act=False)
        emit_chunk(0, 3)
        emit_slab(5, qk_on_act=False)
        emit_chunk(1, 0)
        emit_slab(6, qk_on_act=False)
        emit_outgroup(0, 0)
        emit_chunk(1, 1)
        emit_slab(7, qk_on_act=False)
        emit_outgroup(0, 1)
        emit_chunk(1, 2)
        emit_outgroup(0, 2)
        emit_outgroup(1, 0)
        emit_chunk(1, 3)
        emit_outgroup(0, 3)
        for grp in range(1, 4):
            emit_outgroup(1, grp, act_mod=2)


_NC_CACHE = {}


def _build():
    if "nc" in _NC_CACHE:
        return _NC_CACHE["nc"]
    import concourse.bass as bass
    import concourse.mybir as mybir
    import concourse.tile as tile

    bf16 = mybir.dt.bfloat16
    nc = bass.Bass("TRN2", target_bir_lowering=False, debug=False, num_devices=1)
    d = {
        "xt": nc.dram_tensor("xt", [8, 128, 8, 512], bf16, kind="ExternalInput").ap(),
        "wqT": nc.dram_tensor("wqT", [128, 8, 128], bf16, kind="ExternalInput").ap(),
        "wkT": nc.dram_tensor("wkT", [128, 8, 128], bf16, kind="ExternalInput").ap(),
        "wvT": nc.dram_tensor("wvT", [128, 8, 128], bf16, kind="ExternalInput").ap(),
        "woC": nc.dram_tensor("woC", [128, 1024], bf16, kind="ExternalInput").ap(),
        "cos_t": nc.dram_tensor("cos_t", [128, 2048], bf16, kind="ExternalInput").ap(),
        "sin_t": nc.dram_tensor("sin_t", [128, 2048], bf16, kind="ExternalInput").ap(),
        "tri": nc.dram_tensor("tri", [128, 128], bf16, kind="ExternalInput").ap(),
        "ident": nc.dram_tensor("ident", [128, 128], bf16, kind="ExternalInput").ap(),
        "outp": nc.dram_tensor("outp", [4096, 1024], bf16, kind="ExternalOutput").ap(),
    }
    with tile.TileContext(nc) as tc:
        _emit(nc, tc, d)
    _split_excess_waits(nc)
    _NC_CACHE["nc"] = nc
    return nc


def kernel(x, wq, wk, wv, wo, rope_cos, rope_sin):
    from concourse import bass_utils

    x, wq, wk, wv, wo, rope_cos, rope_sin = (
        np.asarray(a, dtype=np.float32)
        for a in (x, wq, wk, wv, wo, rope_cos, rope_sin)
    )
    in_maps = _make_core_inputs(x, wq, wk, wv, wo, rope_cos, rope_sin)
    nc = _build()
    res = bass_utils.run_bass_kernel_spmd(nc, in_maps, core_ids=list(range(N_CORES)))
    total = np.zeros((B * T, C), np.float32)
    for i in range(N_CORES):
        total += res.results[i]["outp"].astype(np.float32)
    return total.reshape(B, T, C).astype(np.float32)


# revision 8
# speedup vs baseline: 1.5099x; 1.0493x over previous
"""Trainium2 Bass kernel for nn_CausalSelfAttention (BitLinear QKV/O + RoPE + causal attn).

Sharding: 2 heads x 2 batches per core (head-parallel), bf16 throughout.
Per core: q/k/v projections ([d, t] layout, bf16 matmuls, fp32 PSUM), RoPE via
DMA partition shuffle + DVE/Pool elementwise, scores in [k, q] layout, exp on
ACT (scale=0.125) -> E bf16, pv as many small-N matmuls with E as the
stationary operand giving [q, d_aug] output whose 65th column is the softmax
denominator (ones column baked into v_aug), per-partition normalization via
DVE tensor_tensor with a broadcast reciprocal, PE transposes back to [d, t],
column-sharded output projection producing a bf16 partial [4096, 1024] that
the host sums across cores.

Engine budget per core (TimelineSim model): PE ~102us (at the matmul floor:
cost = out-free-size x 0.417ns/row), ACT ~85us (exp + some copies), DVE ~85us
(copies, rope, epilogue), Pool ~45us (tri masks, rope adds). Emission order
interleaves batch-0 attention with batch-1 projections (and batch-1 attention
with batch-0 out-projection) so exp overlaps projection matmuls.
"""
import sys

sys.path.insert(0, "/opt/trn_rl_repo")

import ml_dtypes
import numpy as np

BF16 = ml_dtypes.bfloat16
GROUP = 128
N_HEADS = 16
EPS = 1e-8
B, T, C = 2, 2048, 1024
HD = 64
N_CORES = 8
HPC = N_HEADS // N_CORES  # 2 heads per core


# ---------------------------------------------------------------- host prep
def _ternary_quantize(w):
    O, I = w.shape
    g = w.reshape(O, I // GROUP, GROUP).astype(np.float32)
    scale = np.maximum(np.mean(np.abs(g), axis=-1, keepdims=True), EPS).astype(
        np.float32
    )
    wn = g / scale
    q = np.where(wn > 0.5, 1.0, np.where(wn < -0.5, -1.0, 0.0)).astype(np.float32)
    return (q * scale).reshape(O, I).astype(np.float32)


def _make_core_inputs(x, wq, wk, wv, wo, rope_cos, rope_sin):
    """Returns list of 8 per-core input dicts (bf16 device layouts)."""
    x = np.ascontiguousarray(x.astype(np.float32).reshape(B * T, C))
    wq_q = _ternary_quantize(wq)
    wk_q = _ternary_quantize(wk)
    wv_q = _ternary_quantize(wv)
    wo_q = _ternary_quantize(wo)

    xT = x.T  # [1024 c, 4096 t]
    xt_slab = np.ascontiguousarray(
        xT.reshape(8, 128, 8, 512).transpose(2, 1, 0, 3)
    ).astype(BF16)  # [s, p, kk, u]

    cosT = rope_cos.astype(np.float32).T  # [32, 2048]
    sinT = rope_sin.astype(np.float32).T
    cos_t = np.tile(cosT, (4, 1)).astype(BF16)
    sin_t = np.concatenate([-sinT, sinT, -sinT, sinT], axis=0).astype(BF16)
    tri = (np.arange(128)[None, :] >= np.arange(128)[:, None]).astype(BF16)
    ident = np.eye(128, dtype=np.float32).astype(BF16)

    maps = []
    for core in range(N_CORES):
        r0 = core * HPC * HD
        rows = slice(r0, r0 + HPC * HD)

        def w_lhsT(w_qq):
            wsT = w_qq[rows, :].T  # [1024 in, 128 d]
            return np.ascontiguousarray(
                wsT.reshape(8, 128, 128).transpose(1, 0, 2)
            ).astype(BF16)  # [p, kk, d]

        woc = wo_q[:, rows]  # [1024 o, 128 d]
        maps.append(
            {
                "xt": xt_slab,
                "wqT": w_lhsT(wq_q),
                "wkT": w_lhsT(wk_q),
                "wvT": w_lhsT(wv_q),
                "woC": np.ascontiguousarray(woc.T).astype(BF16),  # [128 d, 1024 o]
                "cos_t": cos_t,
                "sin_t": sin_t,
                "tri": tri,
                "ident": ident,
            }
        )
    return maps


# ---------------------------------------------------------------- BIR post-pass
def _split_excess_waits(nc, max_waits=1):
    """walrus CoreV3 codegen rejects instructions with >1 sem wait; split the
    excess into preceding NoOps on the same engine."""
    import concourse.mybir as mybir

    for f in nc.m.functions:
        for bb in f.blocks:
            insts = bb.instructions
            i = 0
            while i < len(insts):
                ins = insts[i]
                si = ins.sync_info
                if si is not None and si.on_wait and len(si.on_wait) > max_waits:
                    waits = list(si.on_wait)
                    si.on_wait = waits[:max_waits]
                    rest = waits[max_waits:]
                    new_ops = []
                    for j in range(0, len(rest), max_waits):
                        new_ops.append(
                            mybir.InstNoOp(
                                name=nc.get_next_instruction_name(),
                                sync_info=mybir.SyncInfo(
                                    on_wait=rest[j : j + max_waits], on_update=[]
                                ),
                                bass_nofuse=True,
                                engine=ins.engine,
                            )
                        )
                    insts[i:i] = new_ops
                    i += len(new_ops)
                i += 1


# ---------------------------------------------------------------- device kernel
def _emit(nc, tc, d):
    import concourse.mybir as mybir
    from concourse.bass import ds, ts

    f32 = mybir.dt.float32
    bf16 = mybir.dt.bfloat16
    AF = mybir.ActivationFunctionType
    OP = mybir.AluOpType

    with nc.allow_low_precision(
        reason="bf16 activations; fp32 accum in PSUM; 2e-2 rel tol"
    ), tc.tile_pool(name="const", bufs=1) as cp, tc.tile_pool(
        name="persist", bufs=1
    ) as pp, tc.tile_pool(name="xt", bufs=2) as xtp, tc.tile_pool(
        name="sw", bufs=2
    ) as swp, tc.tile_pool(name="tmp", bufs=2) as tmpp, tc.tile_pool(
        name="E", bufs=3
    ) as epool, tc.tile_pool(name="y2n", bufs=2) as y2np, tc.tile_pool(
        name="rc", bufs=4
    ) as rcp, tc.tile_pool(name="scP", bufs=2, space="PSUM") as scp, tc.tile_pool(
        name="ypP", bufs=2, space="PSUM"
    ) as ypp, tc.tile_pool(name="scrP", bufs=2, space="PSUM") as scr:
        # ---- constants
        wq_t = cp.tile([128, 8, 128], bf16)
        nc.sync.dma_start(wq_t[:], d["wqT"])
        wk_t = cp.tile([128, 8, 128], bf16)
        nc.sync.dma_start(wk_t[:], d["wkT"])
        wv_t = cp.tile([128, 8, 128], bf16)
        nc.sync.dma_start(wv_t[:], d["wvT"])
        woC = cp.tile([128, 1024], bf16)
        nc.sync.dma_start(woC[:], d["woC"])
        cos_sb = cp.tile([128, 2048], bf16)
        nc.sync.dma_start(cos_sb[:], d["cos_t"])
        sin_sb = cp.tile([128, 2048], bf16)
        nc.sync.dma_start(sin_sb[:], d["sin_t"])
        tri_t = cp.tile([128, 128], bf16)
        nc.sync.dma_start(tri_t[:], d["tri"])
        id_t = cp.tile([128, 128], bf16)
        nc.sync.dma_start(id_t[:], d["ident"])

        # ---- persistent tensors
        qT = pp.tile([128, 4096], bf16)
        kT = pp.tile([128, 4096], bf16)
        vT = pp.tile([128, 4096], bf16)
        v_aug = pp.tile([128, 32 * 129], bf16)  # per key-block: [v0(64)|1|v1(64)]
        y2T = pp.tile([128, 4096], bf16)
        bo = [pp.tile([128, 16384], bf16, name=f"bo{b}") for b in range(2)]
        # bake the shared ones columns (col 64 of every 129-block)
        nc.gpsimd.memset(v_aug[:], 1.0)

        # ---- phase A granules: xt prefetch, then per-projection granules.
        # q/k granules include RoPE on the slab's columns; the v granule
        # includes the v transposes — so attention on a slab's queries can
        # start right after its three projection granules.
        _xt_tiles = {}

        def g_xt(s):
            def f():
                t = xtp.tile([128, 8, 512], bf16, tag="xt")
                nc.sync.dma_start(t[:], d["xt"][s])
                _xt_tiles[s] = t
            return f

        def _rope(tns, s):
            u = (s % 4) * 512  # within-batch token offset
            ccols = ds(u, 512)
            scols = ds(s * 512, 512)
            sw = swp.tile([128, 512], bf16, tag="sw")
            nc.sync.dma_start(sw[0:32, :], tns[32:64, scols])
            nc.sync.dma_start(sw[32:64, :], tns[0:32, scols])
            nc.sync.dma_start(sw[64:96, :], tns[96:128, scols])
            nc.sync.dma_start(sw[96:128, :], tns[64:96, scols])
            tmp = tmpp.tile([128, 512], bf16, tag="tmp")
            nc.vector.tensor_tensor(tmp[:], tns[:, scols], cos_sb[:, ccols], OP.mult)
            nc.vector.tensor_tensor(sw[:], sw[:], sin_sb[:, ccols], OP.mult)
            nc.gpsimd.tensor_tensor(tns[:, scols], tmp[:], sw[:], OP.add)

        def g_proj(s, which, qk_on_act=False):
            def f():
                xt_t = _xt_tiles[s]
                w_t, dest = ((wq_t, qT), (wk_t, kT), (wv_t, vT))[which]
                ps = scr.tile([128, 512], f32, tag="scr")
                for kk in range(8):
                    nc.tensor.matmul(
                        ps[:],
                        w_t[:, kk, :],
                        xt_t[:, kk, :],
                        start=(kk == 0),
                        stop=(kk == 7),
                    )
                if qk_on_act and which < 2:
                    nc.scalar.copy(dest[:, ts(s, 512)], ps[:])
                else:
                    nc.vector.tensor_copy(dest[:, ts(s, 512)], ps[:])
                if which < 2:
                    _rope(dest, s)
                else:
                    for blk in range(4):
                        g = s * 4 + blk
                        tp = scr.tile([128, 128], bf16, tag="scr")
                        nc.tensor.transpose(
                            tp[:], vT[:, ds(g * 128, 128)], id_t[:]
                        )
                        nc.vector.tensor_copy(
                            v_aug[:, ds(g * 129, 64)], tp[:, 0:64]
                        )
                        nc.vector.tensor_copy(
                            v_aug[:, ds(g * 129 + 65, 64)], tp[:, 64:128]
                        )
            return f

        # ---- filler queue: independent PE work popped between attention
        # pipeline units so the in-order PE stream never starves while ACT
        # works through the exp backlog. Items are (slab_done_marker, fn).
        filler = []

        def pop_filler(n=1):
            for _ in range(n):
                if filler:
                    filler.pop(0)[1]()

        def drain_slab(s):
            while any(m is not None and m <= s for m, _ in filler):
                filler.pop(0)[1]()

        # ---- phase B: one (batch, 512-query-chunk) of attention
        def emit_chunk(b, qi):
            nj = 4 * qi + 4
            npairs = nj // 2
            yp = [ypp.tile([128, 260], f32, tag="yp", name=f"yp{b}_{qi}_{h}")
                  for h in range(2)]
            q0 = b * 2048 + qi * 512

            units = [(p, h) for p in range(npairs) for h in range(2)]

            def emit_sc(p, h):
                sc = scp.tile([128, 1024], f32, tag="sc", name=f"sc{b}_{qi}_{p}_{h}")
                for jj in range(2):
                    j = 2 * p + jj
                    dlt0 = max(j * 128 - qi * 512, 0)
                    nc.tensor.matmul(
                        sc[:, ds(jj * 512 + dlt0, 512 - dlt0)],
                        kT[64 * h : 64 * h + 64, ds(b * 2048 + j * 128, 128)],
                        qT[64 * h : 64 * h + 64, ds(q0 + dlt0, 512 - dlt0)],
                        start=True,
                        stop=True,
                    )
                return sc

            def emit_rest(p, h, sc):
                # exp (trim the diagonal pairs so unwritten PSUM is never read)
                E = epool.tile([128, 1024], bf16, tag="E")
                j0, j1 = 2 * p, 2 * p + 1
                d0 = max(j0 * 128 - qi * 512, 0)
                d1 = max(j1 * 128 - qi * 512, 0)
                if d0 == 0 and d1 == 0:
                    nc.scalar.activation(E[:], sc[:], AF.Exp, scale=0.125)
                else:
                    nc.scalar.activation(
                        E[:, ds(d0, 512 - d0)], sc[:, ds(d0, 512 - d0)],
                        AF.Exp, scale=0.125,
                    )
                    nc.scalar.activation(
                        E[:, ds(512 + d1, 512 - d1)], sc[:, ds(512 + d1, 512 - d1)],
                        AF.Exp, scale=0.125,
                    )
                # triangular mask on diagonal blocks
                for jj, j in ((0, j0), (1, j1)):
                    if j >= 4 * qi:
                        qbl = j - 4 * qi
                        esl = E[:, ds(jj * 512 + qbl * 128, 128)]
                        nc.vector.tensor_tensor(esl, esl, tri_t[:], OP.mult)
                # pv: E block is the stationary operand -> out [q, d_aug]
                for jj, j in ((0, j0), (1, j1)):
                    g = b * 16 + j
                    for qbl in range(4):
                        qb_g = qi * 4 + qbl
                        if qb_g < j:
                            continue
                        nc.tensor.matmul(
                            yp[h][:, ds(qbl * 65, 65)],
                            E[:, ds(jj * 512 + qbl * 128, 128)],
                            v_aug[:, ds(g * 129 + 64 * h, 65)],
                            start=(j == 0),
                            stop=(j == qb_g),
                            skip_group_check=True,
                        )

            prev = None
            for ui, u in enumerate(units):
                sc = emit_sc(*u)
                if prev is not None:
                    emit_rest(prev[0], prev[1], prev[2])
                prev = (u[0], u[1], sc)
                if ui % 2 == 1:
                    pop_filler(1)
            emit_rest(prev[0], prev[1], prev[2])

            # epilogue: normalize by the denominator column, then transpose
            y2n = y2np.tile([128, 512], bf16, tag="y2n")
            for h in range(2):
                ypr = yp[h][:].rearrange("p (a c) -> p a c", a=4)
                rc = rcp.tile([128, 4], f32, tag="rc")
                den = ypr[:, :, 64:65] if h == 0 else ypr[:, :, 0:1]
                nc.vector.reciprocal(rc[:], den)
                data = ypr[:, :, 0:64] if h == 0 else ypr[:, :, 1:65]
                rcb = rc[:].unsqueeze(2).broadcast_to([128, 4, 64])
                outap = y2n[:].rearrange("p (a c) -> p a c", a=4)[
                    :, :, 64 * h : 64 * h + 64
                ]
                nc.vector.tensor_tensor(outap, data, rcb, OP.mult)
            for qbl in range(4):
                tp = scr.tile([128, 128], bf16, tag="scr")
                nc.tensor.transpose(tp[:], y2n[:, ds(qbl * 128, 128)], id_t[:])
                nc.vector.tensor_copy(
                    y2T[:, ds(b * 2048 + (qi * 4 + qbl) * 128, 128)], tp[:]
                )

        # ---- phase C: output projection granules (one token-block each)
        _copy_ctr = [0]

        def g_outblk(b, tb, act_mod):
            def f():
                for oc in range(2):
                    op = scr.tile([128, 512], f32, tag="scr")
                    nc.tensor.matmul(
                        op[:],
                        y2T[:, ds(b * 2048 + tb * 128, 128)],
                        woC[:, ds(oc * 512, 512)],
                        start=True,
                        stop=True,
                    )
                    dst = bo[b][:, ds(tb * 1024 + oc * 512, 512)]
                    _copy_ctr[0] += 1
                    if _copy_ctr[0] % act_mod == 0:
                        nc.scalar.copy(dst, op[:])
                    else:
                        nc.vector.tensor_copy(dst, op[:])
            return f

        def g_outdma(b, grp):
            def f():
                dram = d["outp"][ds(b * 2048 + grp * 512, 512), :].rearrange(
                    "(a p) c -> p a c", p=128
                )
                src = bo[b][:, ds(grp * 4096, 4096)].rearrange(
                    "p (a c) -> p a c", a=4
                )
                nc.sync.dma_start(dram, src)
            return f

        # ---------------- emission schedule ----------------
        # slab s feeds chunk (s//4, s%4); attention starts right after slab 0.
        g_xt(0)()
        g_xt(1)()
        for w in range(3):
            g_proj(0, w, qk_on_act=True)()
        # filler: remaining slabs (xt prefetched one slab ahead) ...
        for s in range(1, 8):
            if s + 1 < 8:
                filler.append((None, g_xt(s + 1)))
            for w in range(3):
                filler.append((s, g_proj(s, w, qk_on_act=(s == 1))))
        # ... then batch-0 out-projection blocks
        for grp in range(4):
            for tb in range(grp * 4, grp * 4 + 4):
                filler.append((None, g_outblk(0, tb, act_mod=4)))
            filler.append((None, g_outdma(0, grp)))

        for qi in range(4):
            drain_slab(qi)
            emit_chunk(0, qi)
        for qi in range(4):
            drain_slab(4 + qi)
            emit_chunk(1, qi)
        pop_filler(len(filler))
        for grp in range(4):
            for tb in range(grp * 4, grp * 4 + 4):
                g_outblk(1, tb, act_mod=2)()
            g_outdma(1, grp)()


_NC_CACHE = {}


def _build():
    if "nc" in _NC_CACHE:
        return _NC_CACHE["nc"]
    import concourse.bass as bass
    import concourse.mybir as mybir
    import concourse.tile as tile

    bf16 = mybir.dt.bfloat16
    nc = bass.Bass("TRN2", target_bir_lowering=False, debug=False, num_devices=1)
    d = {
        "xt": nc.dram_tensor("xt", [8, 128, 8, 512], bf16, kind="ExternalInput").ap(),
        "wqT": nc.dram_tensor("wqT", [128, 8, 128], bf16, kind="ExternalInput").ap(),
        "wkT": nc.dram_tensor("wkT", [128, 8, 128], bf16, kind="ExternalInput").ap(),
        "wvT": nc.dram_tensor("wvT", [128, 8, 128], bf16, kind="ExternalInput").ap(),
        "woC": nc.dram_tensor("woC", [128, 1024], bf16, kind="ExternalInput").ap(),
        "cos_t": nc.dram_tensor("cos_t", [128, 2048], bf16, kind="ExternalInput").ap(),
        "sin_t": nc.dram_tensor("sin_t", [128, 2048], bf16, kind="ExternalInput").ap(),
        "tri": nc.dram_tensor("tri", [128, 128], bf16, kind="ExternalInput").ap(),
        "ident": nc.dram_tensor("ident", [128, 128], bf16, kind="ExternalInput").ap(),
        "outp": nc.dram_tensor("outp", [4096, 1024], bf16, kind="ExternalOutput").ap(),
    }
    with tile.TileContext(nc) as tc:
        _emit(nc, tc, d)
    _split_excess_waits(nc)
    _NC_CACHE["nc"] = nc
    return nc


def kernel(x, wq, wk, wv, wo, rope_cos, rope_sin):
    from concourse import bass_utils

    x, wq, wk, wv, wo, rope_cos, rope_sin = (
        np.asarray(a, dtype=np.float32)
        for a in (x, wq, wk, wv, wo, rope_cos, rope_sin)
    )
    in_maps = _make_core_inputs(x, wq, wk, wv, wo, rope_cos, rope_sin)
    nc = _build()
    res = bass_utils.run_bass_kernel_spmd(nc, in_maps, core_ids=list(range(N_CORES)))
    total = np.zeros((B * T, C), np.float32)
    for i in range(N_CORES):
        total += res.results[i]["outp"].astype(np.float32)
    return total.reshape(B, T, C).astype(np.float32)


# revision 13
# speedup vs baseline: 1.5176x; 1.0051x over previous
"""Trainium2 Bass kernel for nn_CausalSelfAttention (BitLinear QKV/O + RoPE + causal attn).

Sharding: 2 heads x 2 batches per core (head-parallel), bf16 throughout.
Per core: q/k/v projections ([d, t] layout, bf16 matmuls, fp32 PSUM), RoPE via
DMA partition shuffle + DVE/Pool elementwise, scores in [k, q] layout, exp on
ACT (scale=0.125) -> E bf16, pv as many small-N matmuls with E as the
stationary operand giving [q, d_aug] output whose 65th column is the softmax
denominator (ones column baked into v_aug), per-partition normalization via
DVE tensor_tensor with a broadcast reciprocal, PE transposes back to [d, t],
column-sharded output projection producing a bf16 partial [4096, 1024] that
the host sums across cores.

Engine budget per core (TimelineSim model): PE ~102us (at the matmul floor:
cost = out-free-size x 0.417ns/row), ACT ~85us (exp + some copies), DVE ~85us
(copies, rope, epilogue), Pool ~45us (tri masks, rope adds). Emission order
interleaves batch-0 attention with batch-1 projections (and batch-1 attention
with batch-0 out-projection) so exp overlaps projection matmuls.
"""
import sys

sys.path.insert(0, "/opt/trn_rl_repo")

import ml_dtypes
import numpy as np

BF16 = ml_dtypes.bfloat16
GROUP = 128
N_HEADS = 16
EPS = 1e-8
B, T, C = 2, 2048, 1024
HD = 64
N_CORES = 8
HPC = N_HEADS // N_CORES  # 2 heads per core


# ---------------------------------------------------------------- host prep
def _ternary_quantize(w):
    O, I = w.shape
    g = w.reshape(O, I // GROUP, GROUP).astype(np.float32)
    scale = np.maximum(np.mean(np.abs(g), axis=-1, keepdims=True), EPS).astype(
        np.float32
    )
    wn = g / scale
    q = np.where(wn > 0.5, 1.0, np.where(wn < -0.5, -1.0, 0.0)).astype(np.float32)
    return (q * scale).reshape(O, I).astype(np.float32)


def _make_core_inputs(x, wq, wk, wv, wo, rope_cos, rope_sin):
    """Returns list of 8 per-core input dicts (bf16 device layouts)."""
    x = np.ascontiguousarray(x.astype(np.float32).reshape(B * T, C))
    wq_q = _ternary_quantize(wq)
    wk_q = _ternary_quantize(wk)
    wv_q = _ternary_quantize(wv)
    wo_q = _ternary_quantize(wo)

    xT = x.T  # [1024 c, 4096 t]
    xt_slab = np.ascontiguousarray(
        xT.reshape(8, 128, 8, 512).transpose(2, 1, 0, 3)
    ).astype(BF16)  # [s, p, kk, u]

    cosT = rope_cos.astype(np.float32).T  # [32, 2048]
    sinT = rope_sin.astype(np.float32).T
    cos_t = np.tile(cosT, (4, 1)).astype(BF16)
    sin_t = np.concatenate([-sinT, sinT, -sinT, sinT], axis=0).astype(BF16)
    # strict upper triangle (invalid: key k > query q within a diagonal block)
    tri = (np.arange(128)[None, :] < np.arange(128)[:, None]).astype(BF16)
    ident = np.eye(128, dtype=np.float32).astype(BF16)
    negid = (-1000.0 * np.eye(128, dtype=np.float32)).astype(BF16)

    maps = []
    for core in range(N_CORES):
        r0 = core * HPC * HD
        rows = slice(r0, r0 + HPC * HD)

        def w_lhsT(w_qq):
            wsT = w_qq[rows, :].T  # [1024 in, 128 d]
            return np.ascontiguousarray(
                wsT.reshape(8, 128, 128).transpose(1, 0, 2)
            ).astype(BF16)  # [p, kk, d]

        woc = wo_q[:, rows]  # [1024 o, 128 d]
        maps.append(
            {
                "xt": xt_slab,
                "wqT": w_lhsT(wq_q),
                "wkT": w_lhsT(wk_q),
                "wvT": w_lhsT(wv_q),
                "woC": np.ascontiguousarray(woc.T).astype(BF16),  # [128 d, 1024 o]
                "cos_t": cos_t,
                "sin_t": sin_t,
                "tri": tri,
                "ident": ident,
                "negid": negid,
            }
        )
    return maps


# ---------------------------------------------------------------- BIR post-pass
def _split_excess_waits(nc, max_waits=1):
    """walrus CoreV3 codegen rejects instructions with >1 sem wait; split the
    excess into preceding NoOps on the same engine."""
    import concourse.mybir as mybir

    for f in nc.m.functions:
        for bb in f.blocks:
            insts = bb.instructions
            i = 0
            while i < len(insts):
                ins = insts[i]
                si = ins.sync_info
                if si is not None and si.on_wait and len(si.on_wait) > max_waits:
                    waits = list(si.on_wait)
                    si.on_wait = waits[:max_waits]
                    rest = waits[max_waits:]
                    new_ops = []
                    for j in range(0, len(rest), max_waits):
                        new_ops.append(
                            mybir.InstNoOp(
                                name=nc.get_next_instruction_name(),
                                sync_info=mybir.SyncInfo(
                                    on_wait=rest[j : j + max_waits], on_update=[]
                                ),
                                bass_nofuse=True,
                                engine=ins.engine,
                            )
                        )
                    insts[i:i] = new_ops
                    i += len(new_ops)
                i += 1


# ---------------------------------------------------------------- device kernel
def _emit(nc, tc, d):
    import concourse.mybir as mybir
    from concourse.bass import ds, ts

    f32 = mybir.dt.float32
    bf16 = mybir.dt.bfloat16
    AF = mybir.ActivationFunctionType
    OP = mybir.AluOpType

    with nc.allow_low_precision(
        reason="bf16 activations; fp32 accum in PSUM; 2e-2 rel tol"
    ), tc.tile_pool(name="const", bufs=1) as cp, tc.tile_pool(
        name="persist", bufs=1
    ) as pp, tc.tile_pool(name="xt", bufs=3) as xtp, tc.tile_pool(
        name="sw", bufs=2
    ) as swp, tc.tile_pool(name="tmp", bufs=2) as tmpp, tc.tile_pool(
        name="E", bufs=3
    ) as epool, tc.tile_pool(name="y2n", bufs=2) as y2np, tc.tile_pool(
        name="rc", bufs=4
    ) as rcp, tc.tile_pool(name="scP", bufs=2, space="PSUM") as scp, tc.tile_pool(
        name="ypP", bufs=2, space="PSUM"
    ) as ypp, tc.tile_pool(name="scrP", bufs=2, space="PSUM") as scr:
        # ---- constants
        wq_t = cp.tile([128, 8, 128], bf16)
        nc.sync.dma_start(wq_t[:], d["wqT"])
        wk_t = cp.tile([128, 8, 128], bf16)
        nc.sync.dma_start(wk_t[:], d["wkT"])
        wv_t = cp.tile([128, 8, 128], bf16)
        nc.sync.dma_start(wv_t[:], d["wvT"])
        woC = cp.tile([128, 1024], bf16)
        nc.sync.dma_start(woC[:], d["woC"])
        cos_sb = cp.tile([128, 2048], bf16)
        nc.sync.dma_start(cos_sb[:], d["cos_t"])
        sin_sb = cp.tile([128, 2048], bf16)
        nc.sync.dma_start(sin_sb[:], d["sin_t"])
        tri_t = cp.tile([128, 128], bf16)
        nc.sync.dma_start(tri_t[:], d["tri"])
        id_t = cp.tile([128, 128], bf16)
        nc.sync.dma_start(id_t[:], d["ident"])
        nid_t = cp.tile([128, 128], bf16)
        nc.sync.dma_start(nid_t[:], d["negid"])

        # ---- persistent tensors
        qT = pp.tile([128, 4096], bf16)
        kT = pp.tile([128, 4096], bf16)
        vT = pp.tile([128, 4096], bf16)
        v_aug = pp.tile([128, 32 * 129], bf16)  # per key-block: [v0(64)|1|v1(64)]
        y2T = pp.tile([128, 4096], bf16)
        bo = [pp.tile([128, 16384], bf16, name=f"bo{b}") for b in range(2)]
        # bake the shared ones columns (col 64 of every 129-block)
        nc.gpsimd.memset(v_aug[:], 1.0)

        # ---- phase A granules: xt prefetch, then per-projection granules.
        # q/k granules include RoPE on the slab's columns; the v granule
        # includes the v transposes — so attention on a slab's queries can
        # start right after its three projection granules.
        _xt_tiles = {}

        def g_xt(s):
            def f():
                t = xtp.tile([128, 8, 512], bf16, tag="xt")
                nc.sync.dma_start(t[:], d["xt"][s])
                _xt_tiles[s] = t
            return f

        def _rope(tns, s):
            u = (s % 4) * 512  # within-batch token offset
            ccols = ds(u, 512)
            scols = ds(s * 512, 512)
            sw = swp.tile([128, 512], bf16, tag="sw")
            nc.sync.dma_start(sw[0:32, :], tns[32:64, scols])
            nc.sync.dma_start(sw[32:64, :], tns[0:32, scols])
            nc.sync.dma_start(sw[64:96, :], tns[96:128, scols])
            nc.sync.dma_start(sw[96:128, :], tns[64:96, scols])
            tmp = tmpp.tile([128, 512], bf16, tag="tmp")
            nc.vector.tensor_tensor(tmp[:], tns[:, scols], cos_sb[:, ccols], OP.mult)
            nc.vector.tensor_tensor(sw[:], sw[:], sin_sb[:, ccols], OP.mult)
            eng = nc.vector if s == 0 else nc.gpsimd
            eng.tensor_tensor(tns[:, scols], tmp[:], sw[:], OP.add)

        def g_proj(s, which, qk_on_act=False):
            def f():
                xt_t = _xt_tiles[s]
                w_t, dest = ((wq_t, qT), (wk_t, kT), (wv_t, vT))[which]
                ps = scr.tile([128, 512], f32, tag="scr")
                for kk in range(8):
                    nc.tensor.matmul(
                        ps[:],
                        w_t[:, kk, :],
                        xt_t[:, kk, :],
                        start=(kk == 0),
                        stop=(kk == 7),
                    )
                if qk_on_act and which < 2:
                    nc.scalar.copy(dest[:, ts(s, 512)], ps[:])
                else:
                    nc.vector.tensor_copy(dest[:, ts(s, 512)], ps[:])
                if which < 2:
                    _rope(dest, s)
                else:
                    for blk in range(4):
                        g = s * 4 + blk
                        tp = scr.tile([128, 128], bf16, tag="scr")
                        nc.tensor.transpose(
                            tp[:], vT[:, ds(g * 128, 128)], id_t[:]
                        )
                        nc.vector.tensor_copy(
                            v_aug[:, ds(g * 129, 64)], tp[:, 0:64]
                        )
                        nc.vector.tensor_copy(
                            v_aug[:, ds(g * 129 + 65, 64)], tp[:, 64:128]
                        )
            return f

        # ---- filler queue: independent PE work popped between attention
        # pipeline units so the in-order PE stream never starves while ACT
        # works through the exp backlog. Items are (slab_done_marker, fn).
        filler = []

        def pop_filler(n=1):
            for _ in range(n):
                if filler:
                    filler.pop(0)[1]()

        def drain_slab(s):
            while any(m is not None and m <= s for m, _ in filler):
                filler.pop(0)[1]()

        # ---- phase B: one (batch, 512-query-chunk) of attention
        def emit_chunk(b, qi):
            nj = 4 * qi + 4
            npairs = nj // 2
            yp = [ypp.tile([128, 260], f32, tag="yp", name=f"yp{b}_{qi}_{h}")
                  for h in range(2)]
            q0 = b * 2048 + qi * 512

            units = [(p, h) for p in range(npairs) for h in range(2)]

            def emit_sc(p, h):
                sc = scp.tile([128, 1024], f32, tag="sc", name=f"sc{b}_{qi}_{p}_{h}")
                for jj in range(2):
                    j = 2 * p + jj
                    dlt0 = max(j * 128 - qi * 512, 0)
                    nc.tensor.matmul(
                        sc[:, ds(jj * 512 + dlt0, 512 - dlt0)],
                        kT[64 * h : 64 * h + 64, ds(b * 2048 + j * 128, 128)],
                        qT[64 * h : 64 * h + 64, ds(q0 + dlt0, 512 - dlt0)],
                        start=True,
                        stop=True,
                    )
                    if j >= 4 * qi:
                        # causal mask: add -1000 to the strict upper triangle
                        # of the diagonal block so exp() flushes it to zero
                        qbl = j - 4 * qi
                        nc.tensor.matmul(
                            sc[:, ds(jj * 512 + qbl * 128, 128)],
                            nid_t[:],
                            tri_t[:],
                            start=False,
                            stop=True,
                            skip_group_check=True,
                        )
                return sc

            def emit_rest(p, h, sc):
                # exp (trim the diagonal pairs so unwritten PSUM is never read)
                E = epool.tile([128, 1024], bf16, tag="E")
                j0, j1 = 2 * p, 2 * p + 1
                d0 = max(j0 * 128 - qi * 512, 0)
                d1 = max(j1 * 128 - qi * 512, 0)
                if d0 == 0 and d1 == 0:
                    nc.scalar.activation(E[:], sc[:], AF.Exp, scale=0.125)
                else:
                    nc.scalar.activation(
                        E[:, ds(d0, 512 - d0)], sc[:, ds(d0, 512 - d0)],
                        AF.Exp, scale=0.125,
                    )
                    nc.scalar.activation(
                        E[:, ds(512 + d1, 512 - d1)], sc[:, ds(512 + d1, 512 - d1)],
                        AF.Exp, scale=0.125,
                    )
                # pv: E block is the stationary operand -> out [q, d_aug]
                for jj, j in ((0, j0), (1, j1)):
                    g = b * 16 + j
                    for qbl in range(4):
                        qb_g = qi * 4 + qbl
                        if qb_g < j:
                            continue
                        nc.tensor.matmul(
                            yp[h][:, ds(qbl * 65, 65)],
                            E[:, ds(jj * 512 + qbl * 128, 128)],
                            v_aug[:, ds(g * 129 + 64 * h, 65)],
                            start=(j == 0),
                            stop=(j == qb_g),
                            skip_group_check=True,
                        )

            prev = None
            for ui, u in enumerate(units):
                sc = emit_sc(*u)
                if prev is not None:
                    emit_rest(prev[0], prev[1], prev[2])
                prev = (u[0], u[1], sc)
                if ui % 2 == 1:
                    pop_filler(1)
            emit_rest(prev[0], prev[1], prev[2])

            # epilogue: normalize by the denominator column, then transpose
            y2n = y2np.tile([128, 512], bf16, tag="y2n")
            for h in range(2):
                ypr = yp[h][:].rearrange("p (a c) -> p a c", a=4)
                rc = rcp.tile([128, 4], f32, tag="rc")
                den = ypr[:, :, 64:65] if h == 0 else ypr[:, :, 0:1]
                nc.vector.reciprocal(rc[:], den)
                data = ypr[:, :, 0:64] if h == 0 else ypr[:, :, 1:65]
                rcb = rc[:].unsqueeze(2).broadcast_to([128, 4, 64])
                outap = y2n[:].rearrange("p (a c) -> p a c", a=4)[
                    :, :, 64 * h : 64 * h + 64
                ]
                nc.vector.tensor_tensor(outap, data, rcb, OP.mult)
            for qbl in range(4):
                tp = scr.tile([128, 128], bf16, tag="scr")
                nc.tensor.transpose(tp[:], y2n[:, ds(qbl * 128, 128)], id_t[:])
                nc.vector.tensor_copy(
                    y2T[:, ds(b * 2048 + (qi * 4 + qbl) * 128, 128)], tp[:]
                )

        # ---- phase C: output projection granules (one token-block each)
        _copy_ctr = [0]

        def g_outblk(b, tb, act_mod):
            def f():
                for oc in range(2):
                    op = scr.tile([128, 512], f32, tag="scr")
                    nc.tensor.matmul(
                        op[:],
                        y2T[:, ds(b * 2048 + tb * 128, 128)],
                        woC[:, ds(oc * 512, 512)],
                        start=True,
                        stop=True,
                    )
                    dst = bo[b][:, ds(tb * 1024 + oc * 512, 512)]
                    _copy_ctr[0] += 1
                    if _copy_ctr[0] % act_mod == 0:
                        nc.scalar.copy(dst, op[:])
                    else:
                        nc.vector.tensor_copy(dst, op[:])
            return f

        def g_outdma(b, grp):
            def f():
                dram = d["outp"][ds(b * 2048 + grp * 512, 512), :].rearrange(
                    "(a p) c -> p a c", p=128
                )
                src = bo[b][:, ds(grp * 4096, 4096)].rearrange(
                    "p (a c) -> p a c", a=4
                )
                nc.sync.dma_start(dram, src)
            return f

        # ---------------- emission schedule ----------------
        # slab s feeds chunk (s//4, s%4); attention starts right after slab 0.
        g_xt(0)()
        g_xt(1)()
        for w in range(3):
            g_proj(0, w, qk_on_act=True)()
        # filler: remaining slabs (xt prefetched one slab ahead) ...
        for s in range(1, 8):
            if s + 1 < 8:
                filler.append((None, g_xt(s + 1)))
            for w in range(3):
                filler.append((s, g_proj(s, w, qk_on_act=(s == 1))))
        # ... then batch-0 out-projection blocks
        for grp in range(4):
            for tb in range(grp * 4, grp * 4 + 4):
                filler.append((None, g_outblk(0, tb, act_mod=4)))
            filler.append((None, g_outdma(0, grp)))

        for qi in range(4):
            drain_slab(qi)
            emit_chunk(0, qi)
        for qi in range(4):
            drain_slab(4 + qi)
            emit_chunk(1, qi)
            if qi < 3:
                for tb in range(qi * 4, qi * 4 + 4):
                    filler.append((None, g_outblk(1, tb, act_mod=2)))
                filler.append((None, g_outdma(1, qi)))
        pop_filler(len(filler))
        for tb in range(12, 16):
            g_outblk(1, tb, act_mod=2)()
        g_outdma(1, 3)()


_NC_CACHE = {}


def _build():
    if "nc" in _NC_CACHE:
        return _NC_CACHE["nc"]
    import concourse.bass as bass
    import concourse.mybir as mybir
    import concourse.tile as tile

    bf16 = mybir.dt.bfloat16
    nc = bass.Bass("TRN2", target_bir_lowering=False, debug=False, num_devices=1)
    d = {
        "xt": nc.dram_tensor("xt", [8, 128, 8, 512], bf16, kind="ExternalInput").ap(),
        "wqT": nc.dram_tensor("wqT", [128, 8, 128], bf16, kind="ExternalInput").ap(),
        "wkT": nc.dram_tensor("wkT", [128, 8, 128], bf16, kind="ExternalInput").ap(),
        "wvT": nc.dram_tensor("wvT", [128, 8, 128], bf16, kind="ExternalInput").ap(),
        "woC": nc.dram_tensor("woC", [128, 1024], bf16, kind="ExternalInput").ap(),
        "cos_t": nc.dram_tensor("cos_t", [128, 2048], bf16, kind="ExternalInput").ap(),
        "sin_t": nc.dram_tensor("sin_t", [128, 2048], bf16, kind="ExternalInput").ap(),
        "tri": nc.dram_tensor("tri", [128, 128], bf16, kind="ExternalInput").ap(),
        "ident": nc.dram_tensor("ident", [128, 128], bf16, kind="ExternalInput").ap(),
        "negid": nc.dram_tensor("negid", [128, 128], bf16, kind="ExternalInput").ap(),
        "outp": nc.dram_tensor("outp", [4096, 1024], bf16, kind="ExternalOutput").ap(),
    }
    with tile.TileContext(nc) as tc:
        _emit(nc, tc, d)
    _split_excess_waits(nc)
    _NC_CACHE["nc"] = nc
    return nc


def kernel(x, wq, wk, wv, wo, rope_cos, rope_sin):
    from concourse import bass_utils

    x, wq, wk, wv, wo, rope_cos, rope_sin = (
        np.asarray(a, dtype=np.float32)
        for a in (x, wq, wk, wv, wo, rope_cos, rope_sin)
    )
    in_maps = _make_core_inputs(x, wq, wk, wv, wo, rope_cos, rope_sin)
    nc = _build()
    res = bass_utils.run_bass_kernel_spmd(nc, in_maps, core_ids=list(range(N_CORES)))
    total = np.zeros((B * T, C), np.float32)
    for i in range(N_CORES):
        total += res.results[i]["outp"].astype(np.float32)
    return total.reshape(B, T, C).astype(np.float32)
